# revision 18
# baseline (speedup 1.0000x reference)
"""Trainium2 Bass kernel: batched polynomial + Fourier-series point evaluator.

Math: for each point n and each of B=4 times t_b:
    y_poly[b, n]    = sum_{i<4}  poly[n, i] * t_b^i
    y_fourier[b, n] = sum_{k<18} fa[n, k]*cos(w_k t_b) + fb[n, k]*sin(w_k t_b)
(with Fourier bands gated by model_stage).

Because B=4 is tiny, both outputs are one linear map applied to the 40
per-point coefficients:  Y[:, n] = Basis.T @ W[n, :]  with Basis [40, 8]
computed on host (the transcendentals depend only on the 4 scalar times).
The device kernel is a pure streaming matmul over the coefficient tables;
it is HBM-read-bound (reads cap at ~13 GB/s per SDMA engine with all 8
cores streaming; writes run at ~26 GB/s), so the layout halves read bytes:

  - table DRAM [120, COLS] int8, row 40g+i = coeff i of point-group g,
    quantized per point: q = round(W[n,:]/s_n), s_n = max|W[n,:]|/127.
    The host multiplies the outputs by s_n after readback (the whole
    device pipeline is linear in the coefficients), which keeps rel err
    ~5e-3 -- the matmul itself stays fp16 on exact small integers.
  - the ~13 GB/s/engine cap is on the SBUF-WRITE side of the SDMA path
    (paired-NC port mux), so the int8 bytes must land in SBUF as int8:
    plain HWDGE loads (no DMA cast -- that writes fp16-sized bytes and
    wins nothing), then DVE and ACT dequantize int8 -> fp16 in SBUF,
    split by column range so both engines finish together (DVE also
    carries the PSUM->SBUF output copies).
  - in-DMA chunk sizes ramp 1,2,4,8.. banks so compute starts early
    while steady-state per-partition descriptors stay large.
  - 4 matmuls per PSUM bank at tile_position (0, 32j) on disjoint
    32-column strips of the PE array (same [120, 32] stationary basis).
  - PSUM -> SBUF copies cast to fp16 into a [128, cols] out tile; out-DMAs
    write the full 128-partition tile (all 16 SDMA engines) instead of
    8-row slices (2 engines), which previously serialized a ~40 us tail.
"""

import json

import numpy as np

import concourse.bass as bass
import concourse.mybir as mybir
import concourse.tile as tile
from concourse.bass_utils import run_bass_kernel_spmd

# Problem constants (hardcoded per harness contract).
B = 4
N_POINTS = 128 ** 3            # 2097152
N_CORES = 8
NC_PTS = N_POINTS // N_CORES   # 262144 real points per core
KH = 18                        # harmonics
NCOEF = 40                     # 4 poly + 18 cos + 18 sin

GROUPS = 3                     # point-groups stacked in contraction dim (K=120)
JT = 4                         # concurrent col-strip matmuls per PSUM bank
MM_N = 512                     # matmul moving free size (one PSUM bank of fp32)
BANKS = 43                     # PSUM-bank fills per core (43*2048*3 = 264192 pts)
BCOLS = JT * MM_N              # 2048 table columns per bank
COLS = BANKS * BCOLS           # 88064 table columns per core
NP = GROUPS * COLS             # 264192 padded points per core

IN_CHUNKS = (1, 2, 4, 8, 8, 8, 8, 2, 2)   # banks per in-DMA (sum = 43)
OUT_CHUNKS = (16, 16, 8, 2, 1)            # banks per out-DMA (sum = 43)

_CACHED_NC = None
LAST_RESULTS = None            # BassKernelResults of the most recent run


def _build_module():
    nc = bass.Bass()
    dt = mybir.dt

    table = nc.dram_tensor("table", [GROUPS * NCOEF, COLS], dt.int8,
                           kind="ExternalInput")
    basis = nc.dram_tensor("basis", [GROUPS * NCOEF, 32], dt.float16,
                           kind="ExternalInput")
    out_t = nc.dram_tensor("out_t", [128, BANKS * MM_N], dt.float16,
                           kind="ExternalOutput")

    with tile.TileContext(nc) as tc:
        with (
            tc.tile_pool(name="const", bufs=1) as cpool,
            tc.tile_pool(name="qp", bufs=3) as qpool,
            tc.tile_pool(name="inp", bufs=3) as ipool,
            tc.tile_pool(name="psum", bufs=8, space="PSUM") as ppool,
            tc.tile_pool(name="outp", bufs=2) as opool,
        ):
            # Basis goes via the scalar HWDGE ring so the first table chunk
            # is the very first dispatch on sync's ring.
            basis_sb = cpool.tile([GROUPS * NCOEF, 32], dt.float16)
            nc.scalar.dma_start(basis_sb[:, :], basis[:, :])

            out_tile = None
            oc = 0                 # index into OUT_CHUNKS
            ob0 = 0                # first bank of current out tile
            gb = 0                 # global bank index
            for ci, nb in enumerate(IN_CHUNKS):
                q_tile = qpool.tile([GROUPS * NCOEF, nb * BCOLS], dt.int8)
                nc.sync.dma_start(q_tile[:, :],
                                  table[:, gb * BCOLS : (gb + nb) * BCOLS])
                # Dequant int8 -> fp16 in <=4-bank pieces (pipeline grain),
                # split 855:1193 cols/bank so DVE (which also does the
                # PSUM->SBUF copies, 512 cols/bank) and ACT finish together.
                in_tile = ipool.tile([GROUPS * NCOEF, nb * BCOLS], dt.float16)
                for p0 in range(0, nb, 4):
                    pn = min(4, nb - p0)
                    c0, c1 = p0 * BCOLS, (p0 + pn) * BCOLS
                    dv = c0 + pn * 820
                    nc.vector.tensor_copy(in_tile[:, c0:dv], q_tile[:, c0:dv])
                    nc.scalar.copy(in_tile[:, dv:c1], q_tile[:, dv:c1])
                for b in range(nb):
                    if out_tile is None:
                        onb = OUT_CHUNKS[oc]
                        out_tile = opool.tile([128, onb * MM_N], dt.float16)
                        ob0 = gb
                    ps = ppool.tile([128, MM_N], dt.float32)
                    for j in range(JT):
                        nc.tensor.matmul(
                            ps[32 * j : 32 * (j + 1), :],
                            basis_sb[:, :],
                            in_tile[:, b * BCOLS + MM_N * j
                                    : b * BCOLS + MM_N * (j + 1)],
                            start=True,
                            stop=True,
                            tile_position=(0, 32 * j),
                        )
                    w0 = (gb - ob0) * MM_N
                    nc.vector.tensor_copy(
                        out_tile[:, w0 : w0 + MM_N], ps[:, :]
                    )
                    gb += 1
                    if gb - ob0 == OUT_CHUNKS[oc]:
                        # Big out-DMAs dispatch from the otherwise-idle gpsimd
                        # SWDGE (emission overlaps the read stream); the tiny
                        # final ones use sync's fast HWDGE dispatch (the Q7
                        # would add ~11 us of emission to the tail).
                        oeng = nc.gpsimd if OUT_CHUNKS[oc] >= 8 else nc.sync
                        oeng.dma_start(
                            out_t[:, ob0 * MM_N : gb * MM_N], out_tile[:, :]
                        )
                        out_tile = None
                        oc += 1
    return nc


def _dedupe_ldweights(m: dict) -> None:
    """Drop Ldweights instructions that reload the exact same stationary
    operand into the same PE array position as the previously retained one
    (the weights are static in this kernel).  Any waits on a dropped
    Ldweights migrate to the next instruction in the same engine stream."""
    def sig(ins):
        return json.dumps(
            {k: ins.get(k) for k in ("ins", "tile_position", "perf_mode",
                                     "is_transpose", "tile_size")},
            sort_keys=True,
        )

    def fix_block(b):
        last_by_pos = {}
        out = []
        pending_waits = []
        for ins in b.get("instructions", []):
            if ins.get("opcode") == "Ldweights":
                pos = tuple(ins.get("tile_position") or (0, 0))
                s = sig(ins)
                upd = (ins.get("sync_info") or {}).get("on_update", [])
                if last_by_pos.get(pos) == s and not upd:
                    pending_waits.extend(
                        (ins.get("sync_info") or {}).get("on_wait", []))
                    continue
                last_by_pos[pos] = s
            elif pending_waits and ins.get("engine") == "PE":
                si = ins.setdefault("sync_info", {"on_update": [], "on_wait": []})
                si["on_wait"] = pending_waits + si.get("on_wait", [])
                pending_waits = []
            out.append(ins)
        assert not pending_waits
        b["instructions"] = out
        for ch in b.get("blocks", []):
            fix_block(ch)

    for fn in m["functions"]:
        for b in fn.get("blocks", []):
            fix_block(b)


def _legalize_single_wait(bir_bytes: bytes) -> bytes:
    """Split multi-wait instructions: this walrus build's codegen accepts at
    most ONE sync-wait per ISA instruction.  Hoist all but the last wait onto
    NoOps inserted just before the instruction on the same engine stream
    (the sequencer executes them in order, so semantics are preserved)."""
    m = json.loads(bir_bytes)
    _dedupe_ldweights(m)
    n_split = 0

    def fix_block(b):
        nonlocal n_split
        out = []
        for ins in b.get("instructions", []):
            si = ins.get("sync_info")
            waits = (si or {}).get("on_wait", [])
            if len(waits) > 1 and ins.get("engine", "Unassigned") != "Unassigned":
                for w in waits[:-1]:
                    n_split += 1
                    out.append({
                        "debug": ins.get("debug", 0),
                        "engine": ins["engine"],
                        "ins": [],
                        "name": f"{ins['name']}-wsplit{n_split}",
                        "opcode": "NoOp",
                        "outs": [],
                        "sync_info": {"on_update": [], "on_wait": [w]},
                    })
                si["on_wait"] = [waits[-1]]
            out.append(ins)
        b["instructions"] = out
        for ch in b.get("blocks", []):
            fix_block(ch)

    for fn in m["functions"]:
        for b in fn.get("blocks", []):
            fix_block(b)
    return json.dumps(m).encode()


def _get_module():
    global _CACHED_NC
    if _CACHED_NC is None:
        nc = _build_module()
        orig = nc.to_json_bytes
        nc.to_json_bytes = lambda: _legalize_single_wait(orig())
        _CACHED_NC = nc
    return _CACHED_NC


def _host_basis(input_t: np.ndarray, model_stage) -> np.ndarray:
    """Packed stationary weights [120, 32] fp16: col 8g+jj = output jj of
    point-group g (jj 0-3 poly batch, 4-7 fourier batch)."""
    stage = int(model_stage)
    curr = min(stage, 3) if stage >= 0 else 3
    mask = np.zeros(KH, dtype=np.float64)
    for s, e, req in ((0, 3, 1), (3, 9, 2), (9, KH, 3)):
        if curr >= req:
            mask[s:e] = 1.0

    t = np.asarray(input_t, dtype=np.float64)
    Vp = np.stack([t ** i for i in range(4)], axis=0)           # [4, B]
    w = 2.0 * np.pi * np.arange(1, KH + 1, dtype=np.float64)    # [18]
    Cc = np.cos(np.outer(w, t)) * mask[:, None]                 # [18, B]
    Ss = np.sin(np.outer(w, t)) * mask[:, None]                 # [18, B]

    B8 = np.zeros((NCOEF, 8), dtype=np.float64)
    B8[0:4, 0:4] = Vp
    B8[4:22, 4:8] = Cc
    B8[22:40, 4:8] = Ss

    basis = np.zeros((GROUPS * NCOEF, 32), dtype=np.float64)
    for g in range(GROUPS):
        basis[NCOEF * g : NCOEF * (g + 1), 8 * g : 8 * g + 8] = B8
    return basis.astype(np.float16)


def kernel(input_t, poly_coeffs, fourier_a, fourier_b, model_stage):
    global LAST_RESULTS
    input_t = np.asarray(input_t, dtype=np.float32)
    poly_coeffs = np.asarray(poly_coeffs, dtype=np.float32)
    fourier_a = np.asarray(fourier_a, dtype=np.float32)
    fourier_b = np.asarray(fourier_b, dtype=np.float32)
    assert input_t.shape == (B,)
    assert poly_coeffs.shape == (N_POINTS, 4)
    assert fourier_a.shape == (N_POINTS, KH)
    assert fourier_b.shape == (N_POINTS, KH)

    basis = _host_basis(input_t, model_stage)

    # Per-core table [120, COLS] int8: row 40g+i, col c holds quantized
    # coeff i of point g*COLS + c.  Per-point symmetric scale.
    W = np.concatenate([poly_coeffs, fourier_a, fourier_b], axis=1)  # [N, 40]
    scale = np.abs(W).max(axis=1) / 127.0                            # [N]
    scale = np.maximum(scale, 1e-30)
    Wq = np.clip(np.round(W / scale[:, None]), -127, 127).astype(np.int8)
    Wp = np.zeros((N_CORES, NP, NCOEF), dtype=np.int8)
    Wp[:, :NC_PTS] = Wq.reshape(N_CORES, NC_PTS, NCOEF)
    tables = np.ascontiguousarray(
        Wp.reshape(N_CORES, GROUPS, COLS, NCOEF).transpose(0, 1, 3, 2)
    ).reshape(N_CORES, GROUPS * NCOEF, COLS)

    nc = _get_module()
    in_maps = [{"table": tables[c], "basis": basis} for c in range(N_CORES)]
    LAST_RESULTS = run_bass_kernel_spmd(nc, in_maps, core_ids=list(range(N_CORES)))
    results = LAST_RESULTS.results

    outs = []
    for r in results:
        ot = r["out_t"]  # [128, BANKS*512]; row 32j+8g+jj, col 512u+f
        o = ot.reshape(JT, 4, 8, BANKS, MM_N)[:, :GROUPS]  # [j, g, jj, u, f]
        o = o.transpose(2, 1, 3, 0, 4)                     # [jj, g, u, j, f]
        outs.append(o.reshape(8, NP)[:, :NC_PTS].astype(np.float32))
    out = np.concatenate(outs, axis=1)
    out *= scale[None, :].astype(np.float32)
    return out[0:4], out[4:8]


# revision 23
# speedup vs baseline: 1.1264x; 1.1264x over previous
"""Trainium2 Bass kernel: batched polynomial + Fourier-series point evaluator.

Math: for each point n and each of B=4 times t_b:
    y_poly[b, n]    = sum_{i<4}  poly[n, i] * t_b^i
    y_fourier[b, n] = sum_{k<18} fa[n, k]*cos(w_k t_b) + fb[n, k]*sin(w_k t_b)
(with Fourier bands gated by model_stage).

Because B=4 is tiny, both outputs are one linear map applied to the 40
per-point coefficients:  Y[:, n] = Basis.T @ W[n, :]  with Basis [40, 8]
computed on host (the transcendentals depend only on the 4 scalar times).
The device kernel is a pure streaming matmul over the coefficient tables;
it is HBM-read-bound (reads cap at ~13 GB/s per SDMA engine with all 8
cores streaming; writes run at ~26 GB/s), so the layout halves read bytes:

  - table DRAM [120, COLS] int8, row 40g+i = coeff i of point-group g,
    quantized per point: q = round(W[n,:]/s_n), s_n = max|W[n,:]|/127.
    The host multiplies the outputs by s_n after readback (the whole
    device pipeline is linear in the coefficients), which keeps rel err
    ~5e-3 -- the matmul itself stays fp16 on exact small integers.
  - the ~13 GB/s/engine cap is on the SBUF-WRITE side of the SDMA path
    (paired-NC port mux), so the int8 bytes must land in SBUF as int8:
    plain HWDGE loads (no DMA cast -- that writes fp16-sized bytes and
    wins nothing), then DVE and ACT dequantize int8 -> fp16 in SBUF,
    split by column range so both engines finish together (DVE also
    carries the PSUM->SBUF output copies).
  - in-DMA chunk sizes ramp 1,2,4,8.. banks so compute starts early
    while steady-state per-partition descriptors stay large.
  - 4 matmuls per PSUM bank at tile_position (0, 32j) on disjoint
    32-column strips of the PE array (same [120, 32] stationary basis).
  - PSUM -> SBUF copies cast to fp16 into a [128, cols] out tile; out-DMAs
    write the full 128-partition tile (all 16 SDMA engines) instead of
    8-row slices (2 engines), which previously serialized a ~40 us tail.
"""

import json

import numpy as np

import concourse.bass as bass
import concourse.mybir as mybir
import concourse.tile as tile
from concourse.bass_utils import run_bass_kernel_spmd

# Problem constants (hardcoded per harness contract).
B = 4
N_POINTS = 128 ** 3            # 2097152
N_CORES = 8
NC_PTS = N_POINTS // N_CORES   # 262144 real points per core
KH = 18                        # harmonics
NCOEF = 40                     # 4 poly + 18 cos + 18 sin

GROUPS = 3                     # point-groups stacked in contraction dim (K=120)
JT = 4                         # concurrent col-strip matmuls per PSUM bank
MM_N = 512                     # matmul moving free size (one PSUM bank of fp32)
BANKS = 43                     # PSUM-bank fills per core (43*2048*3 = 264192 pts)
BCOLS = JT * MM_N              # 2048 table columns per bank
COLS = BANKS * BCOLS           # 88064 table columns per core
NP = GROUPS * COLS             # 264192 padded points per core

IN_CHUNKS = (1, 2, 4, 8, 8, 8, 8, 4)   # banks per in-DMA (sum = 43)
OUT_CHUNKS = (16, 16, 8, 2, 1)         # banks per out-DMA (sum = 43)

_CACHED_NC = None
LAST_RESULTS = None            # BassKernelResults of the most recent run


def _build_module():
    nc = bass.Bass()
    dt = mybir.dt

    table = nc.dram_tensor("table", [GROUPS * NCOEF, COLS], dt.int8,
                           kind="ExternalInput")
    basis = nc.dram_tensor("basis", [GROUPS * NCOEF, 32], dt.float16,
                           kind="ExternalInput")
    out_t = nc.dram_tensor("out_t", [128, BANKS * MM_N], dt.float16,
                           kind="ExternalOutput")

    with tile.TileContext(nc) as tc:
        with (
            tc.tile_pool(name="const", bufs=1) as cpool,
            tc.tile_pool(name="qp", bufs=3) as qpool,
            tc.tile_pool(name="inp", bufs=2) as ipool,
            tc.tile_pool(name="psum", bufs=8, space="PSUM") as ppool,
            tc.tile_pool(name="outp", bufs=2) as opool,
        ):
            basis_sb = cpool.tile([GROUPS * NCOEF, 32], dt.float16)
            nc.sync.dma_start(basis_sb[:, :], basis[:, :])

            out_tile = None
            oc = 0                 # index into OUT_CHUNKS
            ob0 = 0                # first bank of current out tile
            gb = 0                 # global bank index
            for ci, nb in enumerate(IN_CHUNKS):
                q_tile = qpool.tile([GROUPS * NCOEF, nb * BCOLS], dt.int8)
                nc.sync.dma_start(q_tile[:, :],
                                  table[:, gb * BCOLS : (gb + nb) * BCOLS])
                # Dequant int8 -> fp16, split so DVE (which also does the
                # PSUM->SBUF copies, 512 cols/bank) and ACT finish together.
                in_tile = ipool.tile([GROUPS * NCOEF, nb * BCOLS], dt.float16)
                dv = nb * 768
                nc.vector.tensor_copy(in_tile[:, :dv], q_tile[:, :dv])
                nc.scalar.copy(in_tile[:, dv:], q_tile[:, dv:])
                for b in range(nb):
                    if out_tile is None:
                        onb = OUT_CHUNKS[oc]
                        out_tile = opool.tile([128, onb * MM_N], dt.float16)
                        ob0 = gb
                    ps = ppool.tile([128, MM_N], dt.float32)
                    for j in range(JT):
                        nc.tensor.matmul(
                            ps[32 * j : 32 * (j + 1), :],
                            basis_sb[:, :],
                            in_tile[:, b * BCOLS + MM_N * j
                                    : b * BCOLS + MM_N * (j + 1)],
                            start=True,
                            stop=True,
                            tile_position=(0, 32 * j),
                        )
                    w0 = (gb - ob0) * MM_N
                    nc.vector.tensor_copy(
                        out_tile[:, w0 : w0 + MM_N], ps[:, :]
                    )
                    gb += 1
                    if gb - ob0 == OUT_CHUNKS[oc]:
                        nc.scalar.dma_start(
                            out_t[:, ob0 * MM_N : gb * MM_N], out_tile[:, :]
                        )
                        out_tile = None
                        oc += 1
    return nc


def _dedupe_ldweights(m: dict) -> None:
    """Drop Ldweights instructions that reload the exact same stationary
    operand into the same PE array position as the previously retained one
    (the weights are static in this kernel).  Any waits on a dropped
    Ldweights migrate to the next instruction in the same engine stream."""
    def sig(ins):
        return json.dumps(
            {k: ins.get(k) for k in ("ins", "tile_position", "perf_mode",
                                     "is_transpose", "tile_size")},
            sort_keys=True,
        )

    def fix_block(b):
        last_by_pos = {}
        out = []
        pending_waits = []
        for ins in b.get("instructions", []):
            if ins.get("opcode") == "Ldweights":
                pos = tuple(ins.get("tile_position") or (0, 0))
                s = sig(ins)
                upd = (ins.get("sync_info") or {}).get("on_update", [])
                if last_by_pos.get(pos) == s and not upd:
                    pending_waits.extend(
                        (ins.get("sync_info") or {}).get("on_wait", []))
                    continue
                last_by_pos[pos] = s
            elif pending_waits and ins.get("engine") == "PE":
                si = ins.setdefault("sync_info", {"on_update": [], "on_wait": []})
                si["on_wait"] = pending_waits + si.get("on_wait", [])
                pending_waits = []
            out.append(ins)
        assert not pending_waits
        b["instructions"] = out
        for ch in b.get("blocks", []):
            fix_block(ch)

    for fn in m["functions"]:
        for b in fn.get("blocks", []):
            fix_block(b)


def _legalize_single_wait(bir_bytes: bytes) -> bytes:
    """Split multi-wait instructions: this walrus build's codegen accepts at
    most ONE sync-wait per ISA instruction.  Hoist all but the last wait onto
    NoOps inserted just before the instruction on the same engine stream
    (the sequencer executes them in order, so semantics are preserved)."""
    m = json.loads(bir_bytes)
    _dedupe_ldweights(m)
    n_split = 0

    def fix_block(b):
        nonlocal n_split
        out = []
        for ins in b.get("instructions", []):
            si = ins.get("sync_info")
            waits = (si or {}).get("on_wait", [])
            if len(waits) > 1 and ins.get("engine", "Unassigned") != "Unassigned":
                for w in waits[:-1]:
                    n_split += 1
                    out.append({
                        "debug": ins.get("debug", 0),
                        "engine": ins["engine"],
                        "ins": [],
                        "name": f"{ins['name']}-wsplit{n_split}",
                        "opcode": "NoOp",
                        "outs": [],
                        "sync_info": {"on_update": [], "on_wait": [w]},
                    })
                si["on_wait"] = [waits[-1]]
            out.append(ins)
        b["instructions"] = out
        for ch in b.get("blocks", []):
            fix_block(ch)

    for fn in m["functions"]:
        for b in fn.get("blocks", []):
            fix_block(b)
    return json.dumps(m).encode()


def _get_module():
    global _CACHED_NC
    if _CACHED_NC is None:
        nc = _build_module()
        orig = nc.to_json_bytes
        nc.to_json_bytes = lambda: _legalize_single_wait(orig())
        _CACHED_NC = nc
    return _CACHED_NC


def _host_basis(input_t: np.ndarray, model_stage) -> np.ndarray:
    """Packed stationary weights [120, 32] fp16: col 8g+jj = output jj of
    point-group g (jj 0-3 poly batch, 4-7 fourier batch)."""
    stage = int(model_stage)
    curr = min(stage, 3) if stage >= 0 else 3
    mask = np.zeros(KH, dtype=np.float64)
    for s, e, req in ((0, 3, 1), (3, 9, 2), (9, KH, 3)):
        if curr >= req:
            mask[s:e] = 1.0

    t = np.asarray(input_t, dtype=np.float64)
    Vp = np.stack([t ** i for i in range(4)], axis=0)           # [4, B]
    w = 2.0 * np.pi * np.arange(1, KH + 1, dtype=np.float64)    # [18]
    Cc = np.cos(np.outer(w, t)) * mask[:, None]                 # [18, B]
    Ss = np.sin(np.outer(w, t)) * mask[:, None]                 # [18, B]

    B8 = np.zeros((NCOEF, 8), dtype=np.float64)
    B8[0:4, 0:4] = Vp
    B8[4:22, 4:8] = Cc
    B8[22:40, 4:8] = Ss

    basis = np.zeros((GROUPS * NCOEF, 32), dtype=np.float64)
    for g in range(GROUPS):
        basis[NCOEF * g : NCOEF * (g + 1), 8 * g : 8 * g + 8] = B8
    return basis.astype(np.float16)


def kernel(input_t, poly_coeffs, fourier_a, fourier_b, model_stage):
    global LAST_RESULTS
    input_t = np.asarray(input_t, dtype=np.float32)
    poly_coeffs = np.asarray(poly_coeffs, dtype=np.float32)
    fourier_a = np.asarray(fourier_a, dtype=np.float32)
    fourier_b = np.asarray(fourier_b, dtype=np.float32)
    assert input_t.shape == (B,)
    assert poly_coeffs.shape == (N_POINTS, 4)
    assert fourier_a.shape == (N_POINTS, KH)
    assert fourier_b.shape == (N_POINTS, KH)

    basis = _host_basis(input_t, model_stage)

    # Per-core table [120, COLS] int8: row 40g+i, col c holds quantized
    # coeff i of point g*COLS + c.  Per-point symmetric scale.
    W = np.concatenate([poly_coeffs, fourier_a, fourier_b], axis=1)  # [N, 40]
    scale = np.abs(W).max(axis=1) / 127.0                            # [N]
    scale = np.maximum(scale, 1e-30)
    Wq = np.clip(np.round(W / scale[:, None]), -127, 127).astype(np.int8)
    Wp = np.zeros((N_CORES, NP, NCOEF), dtype=np.int8)
    Wp[:, :NC_PTS] = Wq.reshape(N_CORES, NC_PTS, NCOEF)
    tables = np.ascontiguousarray(
        Wp.reshape(N_CORES, GROUPS, COLS, NCOEF).transpose(0, 1, 3, 2)
    ).reshape(N_CORES, GROUPS * NCOEF, COLS)

    nc = _get_module()
    in_maps = [{"table": tables[c], "basis": basis} for c in range(N_CORES)]
    LAST_RESULTS = run_bass_kernel_spmd(nc, in_maps, core_ids=list(range(N_CORES)))
    results = LAST_RESULTS.results

    outs = []
    for r in results:
        ot = r["out_t"]  # [128, BANKS*512]; row 32j+8g+jj, col 512u+f
        o = ot.reshape(JT, 4, 8, BANKS, MM_N)[:, :GROUPS]  # [j, g, jj, u, f]
        o = o.transpose(2, 1, 3, 0, 4)                     # [jj, g, u, j, f]
        outs.append(o.reshape(8, NP)[:, :NC_PTS].astype(np.float32))
    out = np.concatenate(outs, axis=1)
    out *= scale[None, :].astype(np.float32)
    return out[0:4], out[4:8]
